# revision 1
# baseline (speedup 1.0000x reference)
"""Trainium2 Bass kernel for nn_DilatedMHCABlock (dilated multi-head self-attention).

Strategy (self-contained; shapes hardcoded for B=4, N=2048, D=1024, H=16,
dh=64, K_WIN=8, DILATION=4):

- The dilated mask  |j-i| <= 32 and (j-i) % 4 == 0  decomposes the sequence
  into 4 independent interleaved subsequences per batch element (residue mod
  4), each of length 512 with a plain +-8 banded attention.  16 subsequences
  total are sharded 2-per-core across 8 NeuronCores -> zero halo exchange,
  no collectives.
- All matmuls run in float32r (TF32-like, full PE rate at free-dim >= 256).
- Activations live in transposed [feature, token] layout so QK-norm, scores,
  AV and both projections need no on-device transposes:
    qT/kT: [d, t]; v: [t, d]; scores computed directly transposed [kw, q].
- exp() needs no max-subtraction (QK-normalized scores are in [-1, 1]).
  inv_nk is applied as the per-partition ACT scale; inv_nq is folded into
  qT via a K=2 indicator matmul broadcast; softmax normalization is deferred
  past AV and applied as a K=1 broadcast matmul + multiply; the V bias is
  folded into the output bias on the host (bo_eff = bo + Wo @ bv).
"""
import sys

sys.path.insert(0, "/opt/trn_rl_repo")

import numpy as np

import bass_rust
import concourse.bass as bass
import concourse.mybir as mybir
import concourse.tile as tile

F32 = mybir.dt.float32
F32R = mybir.dt.float32r
NEG = -1e30
EPS = 1e-6
N_CORES = 8


# ---------------------------------------------------------------------------
# walrus wait legalization: this container's walrus accepts at most 1 sync
# wait per instruction (2 on EventSemaphore).  Tile occasionally attaches
# more; split the excess onto standalone EventSemaphore insts.
_wait_counter = [0]


def _legalize_waits(nc):
    f = nc.m.functions[0]
    for blk in f.blocks:
        insts = blk.instructions
        out = []
        changed = False
        for inst in insts:
            si = inst.sync_info
            waits = list(si.on_wait) if si is not None else []
            cap = 2 if isinstance(inst, mybir.InstEventSemaphore) else 1
            if len(waits) > cap:
                extra, keep = waits[:-cap], waits[-cap:]
                for i in range(0, len(extra), 2):
                    es = mybir.InstEventSemaphore(
                        name=f"wait_split_{_wait_counter[0]}", ins=[], outs=[]
                    )
                    _wait_counter[0] += 1
                    es.engine = inst.engine
                    es.sync_info = bass_rust.SyncInfo(
                        on_wait=extra[i : i + 2], on_update=[]
                    )
                    out.append(es)
                si.on_wait = keep
                changed = True
            out.append(inst)
        if changed:
            blk.instructions = out


def _mm(nc, out, lhsT, rhs, **kw):
    nc.tensor.matmul(out, lhsT.bitcast(F32R), rhs.bitcast(F32R), **kw)


# ---------------------------------------------------------------------------
def _build_nc(phases=5):
    nc = bass.Bass()

    xT_d = nc.declare_dram_parameter("xT", [1024, 1024], F32R, isOutput=False)
    wq_d = nc.declare_dram_parameter("wqT", [1024, 1024], F32R, isOutput=False)
    wk_d = nc.declare_dram_parameter("wkT", [1024, 1024], F32R, isOutput=False)
    wv_d = nc.declare_dram_parameter("wvT", [1024, 1024], F32R, isOutput=False)
    wo_d = nc.declare_dram_parameter("woT", [1024, 1024], F32R, isOutput=False)
    bq_d = nc.declare_dram_parameter("bq", [1024], F32, isOutput=False)
    bk_d = nc.declare_dram_parameter("bk", [1024], F32, isOutput=False)
    bo_d = nc.declare_dram_parameter("bo", [1024], F32, isOutput=False)
    mask_d = nc.declare_dram_parameter("masks", [128, 1024], F32, isOutput=False)
    i16_d = nc.declare_dram_parameter("ind16", [8, 16, 128], F32R, isOutput=False)
    ih_d = nc.declare_dram_parameter("ind_h", [128, 2], F32R, isOutput=False)
    on_d = nc.declare_dram_parameter("ones_c", [128, 1], F32R, isOutput=False)
    out_d = nc.declare_dram_parameter("outT", [1024, 1024], F32, isOutput=True)

    with tile.TileContext(nc) as tc, nc.allow_low_precision(
        reason="float32r tiles are bit-identical to float32"
    ):
        _emit(nc, tc, xT_d, wq_d, wk_d, wv_d, wo_d, bq_d, bk_d, bo_d, mask_d,
              i16_d, ih_d, on_d, out_d, phases)

    _legalize_waits(nc)
    return nc


def _emit(nc, tc, xT_d, wq_d, wk_d, wv_d, wo_d, bq_d, bk_d, bo_d, mask_d,
          i16_d, ih_d, on_d, out_d, phases=5):
    from contextlib import ExitStack

    ctx = ExitStack()
    with ctx:
        p_const = ctx.enter_context(tc.tile_pool(name="const", bufs=1))
        p_xT = ctx.enter_context(tc.tile_pool(name="xT", bufs=8))
        p_qkT = ctx.enter_context(tc.tile_pool(name="qkT", bufs=32))
        p_v = ctx.enter_context(tc.tile_pool(name="v", bufs=8))
        p_w = ctx.enter_context(tc.tile_pool(name="wstream", bufs=2))
        p_wv = ctx.enter_context(tc.tile_pool(name="wvstr", bufs=3))
        p_sq = ctx.enter_context(tc.tile_pool(name="sq", bufs=3))
        p_exp = ctx.enter_context(tc.tile_pool(name="exp", bufs=4))
        p_nkT = ctx.enter_context(tc.tile_pool(name="nkT", bufs=8))
        p_invq = ctx.enter_context(tc.tile_pool(name="invq", bufs=2))
        p_sums = ctx.enter_context(tc.tile_pool(name="sums", bufs=2))
        p_stg = ctx.enter_context(tc.tile_pool(name="stg", bufs=2))
        p_out = ctx.enter_context(tc.tile_pool(name="outst", bufs=2))
        pp = ctx.enter_context(tc.tile_pool(name="pp", bufs=1, space="PSUM"))

        # ---- constants -----------------------------------------------------
        bq_sb = p_const.tile([128, 8], F32, tag="bq")
        bk_sb = p_const.tile([128, 8], F32, tag="bk")
        bo_sb = p_const.tile([128, 8], F32, tag="bo")
        nc.sync.dma_start(out=bq_sb, in_=bq_d.rearrange("(i p) -> p i", p=128))
        nc.sync.dma_start(out=bk_sb, in_=bk_d.rearrange("(i p) -> p i", p=128))
        nc.sync.dma_start(out=bo_sb, in_=bo_d.rearrange("(i p) -> p i", p=128))

        m_sb = p_const.tile([128, 1024], F32, tag="masks")
        nc.sync.dma_start(out=m_sb, in_=mask_d[:, :])
        m_bc = m_sb[:, 0:512]
        m_e = m_sb[:, 512:768]
        m_a0 = m_sb[0:32, 768:1024]

        ind_h = p_const.tile([128, 2], F32R, tag="ind_h")
        nc.sync.dma_start(out=ind_h, in_=ih_d[:, :])
        ind16 = []
        for i in range(8):
            t = p_const.tile([16, 128], F32R, tag=f"ind16_{i}", name=f"ind16_{i}")
            nc.sync.dma_start(out=t, in_=i16_d[i])
            ind16.append(t)

        xT = []
        for j in range(8):
            t = p_xT.tile([128, 1024], F32R, tag="xT", name=f"xT{j}")
            nc.sync.dma_start(out=t, in_=xT_d[128 * j : 128 * j + 128, :])
            xT.append(t)

        # ---- projections + norms ------------------------------------------
        qt = [[None, None] for _ in range(8)]
        kt = [[None, None] for _ in range(8)]
        v = [None] * 8
        nkT = [None] * 8
        nkT_lo = [None] * 8
        inv_nq = [None, None]

        # q, k projections (batched weight fetch, per-chunk eviction)
        for w_d, bias_sb, dst in ((wq_d, bq_sb, qt), (wk_d, bk_sb, kt)):
            for i in range(8):
                ps = [pp.tile([128, 512], F32, tag="projv", bufs=2, name=f"ps1_{_c}") for _c in range(2)]
                wt = p_w.tile([128, 8, 128], F32R, tag="wstream")
                nc.sync.dma_start(
                    out=wt,
                    in_=w_d.rearrange("(j p) o -> p j o", p=128)[
                        :, :, 128 * i : 128 * i + 128
                    ],
                )
                for j in range(8):
                    for c in range(2):
                        _mm(nc, ps[c], wt[:, j, :],
                            xT[j][:, 512 * c : 512 * c + 512],
                            start=(j == 0), stop=(j == 7))
                for c in range(2):
                    t = p_qkT.tile([128, 512], F32R, tag="qkT")
                    nc.vector.tensor_scalar_add(t, ps[c], bias_sb[:, i : i + 1])
                    dst[i][c] = t

        # q norms + pre-normalize q (per chunk)
        for c in range(2):
            inv_nq[c] = p_invq.tile([16, 512], F32R, tag="invq", name=f"invq{c}")
        for c in range(2):
            for i in range(8):
                sq = p_sq.tile([128, 512], F32R, tag="sq")
                nc.vector.tensor_mul(sq, qt[i][c], qt[i][c])
                pnq = pp.tile([2, 512], F32, tag="mA", bufs=2, name="pnq")
                _mm(nc, pnq, ind_h, sq, start=True, stop=True)
                stg = p_stg.tile([2, 512], F32R, tag="stgq", name="stgq")
                nc.scalar.activation(
                    out=stg, in_=pnq, func=mybir.ActivationFunctionType.Sqrt
                )
                nc.sync.dma_start(out=inv_nq[c][2 * i : 2 * i + 2, :], in_=stg)
            nc.vector.tensor_scalar_add(inv_nq[c], inv_nq[c], EPS)
            nc.vector.reciprocal(inv_nq[c], inv_nq[c])
            for i in range(8):
                pb = pp.tile([128, 512], F32, tag="projv", bufs=2, name="pbq")
                _mm(nc, pb, ind16[i], inv_nq[c], start=True, stop=True)
                nc.vector.tensor_mul(qt[i][c], qt[i][c], pb)

        # k norms (direct [t, h] layout, chunk-major)
        for c in range(2):
            pnk = pp.tile([128, 64], F32, tag="mB", bufs=2, name=f"pnk{c}")
            for i in range(8):
                sqk = p_sq.tile([128, 512], F32R, tag="sq")
                nc.vector.tensor_mul(sqk, kt[i][c], kt[i][c])
                for gl in range(4):
                    _mm(nc,
                        pnk[:, 16 * gl + 2 * i : 16 * gl + 2 * i + 2],
                        sqk[:, 128 * gl : 128 * gl + 128],
                        ind_h, start=True, stop=True)
            for gl in range(4):
                g = 4 * c + gl
                t = p_nkT.tile([128, 16], F32, tag="nkT", name=f"nkT{g}")
                nc.scalar.activation(
                    t, pnk[:, 16 * gl : 16 * gl + 16],
                    func=mybir.ActivationFunctionType.Sqrt)
                nc.vector.tensor_scalar_add(t, t, EPS)
                nc.vector.reciprocal(t, t)
                nkT[g] = t
        for g in range(8):
            t = p_nkT.tile([32, 16], F32, tag=f"nkTlo{g}", name=f"nkTlo{g}")
            nc.sync.dma_start(out=t, in_=nkT[g][96:128, :])
            nkT_lo[g] = t

        # v projection with interleaved ones columns: v[g] is [128, 16*65];
        # head h occupies cols [65h, 65h+64), col 65h+64 is all-ones.
        for ghalf in range(2):
            gs = list(range(4 * ghalf, 4 * ghalf + 4))
            for co in range(2):
                psv = {}
                for gi, g in enumerate(gs):
                    tag = "projv" if gi < 2 else "ppbc"
                    psv[g] = pp.tile([128, 512], F32, tag=tag, bufs=2,
                                     name=f"psv{g}_{co}")
                for j in range(8):
                    wvt = p_wv.tile([128, 512], F32R, tag="wvstr")
                    nc.sync.dma_start(
                        out=wvt,
                        in_=wv_d[128 * j : 128 * j + 128, 512 * co : 512 * co + 512],
                    )
                    for g in gs:
                        _mm(nc, psv[g],
                            xT[j][:, 128 * g : 128 * g + 128],
                            wvt, start=(j == 0), stop=(j == 7))
                for g in gs:
                    if v[g] is None:
                        v[g] = p_v.tile([128, 1040], F32R, tag="v", name=f"v{g}")
                        ones_dst = bass.AP(
                            tensor=v[g].tensor, offset=v[g].offset + 64,
                            ap=[[v[g].ap[0][0], 128], [65, 16], [1, 1]],
                        )
                        nc.sync.dma_start(
                            out=ones_dst,
                            in_=bass.AP(tensor=on_d, offset=0,
                                        ap=[[1, 128], [0, 16], [1, 1]]),
                        )
                    dst = bass.AP(
                        tensor=v[g].tensor,
                        offset=v[g].offset + 65 * 8 * co,
                        ap=[[v[g].ap[0][0], 128], [65, 8], [1, 64]],
                    )
                    nc.vector.tensor_copy(out=dst, in_=psv[g])

        if phases <= 2:
            nc.gpsimd.dma_start(out=out_d[0:128, 0:512], in_=qt[0][0])
            return

        # ---- attention -----------------------------------------------------
        atb = [p_xT.tile([128, 1024], F32R, tag="xT", name=f"atb{_i}") for _i in range(8)]
        at = [[atb[_i][:, 0:512], atb[_i][:, 512:1024]] for _i in range(8)]
        sums = [None, None]
        for c in range(2):
            sums[c] = p_sums.tile([16, 512], F32R, tag="sums", name=f"sums{c}")
        for s in range(2):
            for p in range(2):
                t0l = 256 * p
                g = 4 * s + 2 * p
                for h in range(16):
                    hp = 64 * (h % 2)
                    ht = h // 2
                    Q = qt[ht][s][hp : hp + 64, t0l : t0l + 256]
                    kB = kt[ht][s][hp : hp + 64, 128 * (2 * p) : 128 * (2 * p) + 128]
                    kC = kt[ht][s][
                        hp : hp + 64, 128 * (2 * p + 1) : 128 * (2 * p + 1) + 128
                    ]
                    pbc = pp.tile([128, 512], F32, tag="ppbc", bufs=2, name="pbc")
                    _mm(nc, pbc[:, 0:256], kB, Q, start=True, stop=True,
                        tile_position=(hp, 0))
                    _mm(nc, pbc[:, 256:512], kC, Q, start=True, stop=True,
                        tile_position=(hp, 0))
                    exbc = p_exp.tile([128, 512], F32R, tag="expbc", name="exbc")
                    nc.scalar.activation(
                        out=exbc[:, 0:256], in_=pbc[:, 0:256],
                        func=mybir.ActivationFunctionType.Exp,
                        scale=nkT[g][0:128, h : h + 1])
                    nc.scalar.activation(
                        out=exbc[:, 256:512], in_=pbc[:, 256:512],
                        func=mybir.ActivationFunctionType.Exp,
                        scale=nkT[g + 1][0:128, h : h + 1])
                    nc.vector.tensor_mul(exbc, exbc, m_bc)
                    if p == 0:
                        ge, er0, scl, mr0 = g + 2, 0, nkT[g + 2][0:32, h : h + 1], 0
                    else:
                        ge, er0, scl, mr0 = g - 1, 96, nkT_lo[g - 1][:, h : h + 1], 32
                    pse = pp.tile([32, 256], F32, tag="mA", bufs=2, name="pse")
                    kE = kt[ht][s][
                        hp : hp + 64,
                        128 * (ge % 4) + er0 : 128 * (ge % 4) + er0 + 32,
                    ]
                    _mm(nc, pse, kE, Q, start=True, stop=True,
                        tile_position=(hp, 0))
                    exe = p_exp.tile([128, 256], F32R, tag="expe", name="exe")
                    if er0 == 0:
                        nc.scalar.activation(
                            out=exe[0:32, :], in_=pse,
                            func=mybir.ActivationFunctionType.Exp, scale=scl)
                        nc.vector.tensor_mul(
                            exe[0:32, :], exe[0:32, :], m_e[mr0 : mr0 + 32, :])
                    else:
                        exs = p_stg.tile([32, 256], F32R, tag="exstg", name="exstg")
                        nc.scalar.activation(
                            out=exs, in_=pse,
                            func=mybir.ActivationFunctionType.Exp, scale=scl)
                        nc.vector.tensor_mul(exs, exs, m_a0)
                        nc.sync.dma_start(out=exe[96:128, :], in_=exs)
                    # AV with ones-column: psum rows 0:64 = output, row 64 = sums
                    po = pp.tile([65, 256], F32, tag="mB", bufs=2, name="po")
                    av_pieces = [
                        (v[g][0:128, 65 * h : 65 * h + 65], exbc[:, 0:256], 0),
                        (v[g + 1][0:128, 65 * h : 65 * h + 65], exbc[:, 256:512], 0),
                        (v[ge][er0 : er0 + 32, 65 * h : 65 * h + 65],
                         exe[er0 : er0 + 32, :], er0),
                    ]
                    for idx, (vv, ee, r0) in enumerate(av_pieces):
                        _mm(nc, po, vv, ee, start=(idx == 0), stop=(idx == 2),
                            tile_position=(r0, 0))
                    if hp == 0:
                        nc.vector.tensor_copy(
                            out=at[ht][s][0:64, t0l : t0l + 256], in_=po[0:64, :])
                    else:
                        stgo = p_stg.tile([64, 256], F32R, tag="stgo", name="stgo")
                        nc.vector.tensor_copy(out=stgo, in_=po[0:64, :])
                        nc.sync.dma_start(
                            out=at[ht][s][64:128, t0l : t0l + 256], in_=stgo)
                    stgs = p_stg.tile([128, 256], F32R, tag="stgs", name="stgs")
                    nc.vector.tensor_copy(out=stgs[64:65, :], in_=po[64:65, :])
                    nc.sync.dma_start(
                        out=sums[s][h : h + 1, t0l : t0l + 256], in_=stgs[64:65, :])

        if phases <= 3:
            nc.gpsimd.dma_start(out=out_d[0:128, 0:512], in_=at[0][0])
            return

        # ---- deferred softmax normalization --------------------------------
        for c in range(2):
            nc.vector.reciprocal(sums[c], sums[c])
        for ht in range(8):
            for c in range(2):
                pb = pp.tile([128, 512], F32, tag="ppbc", bufs=2, name="pb4")
                _mm(nc, pb, ind16[ht], sums[c], start=True, stop=True)
                nc.vector.tensor_mul(at[ht][c], at[ht][c], pb)

        # ---- output projection ---------------------------------------------
        for i in range(8):
            wt = p_w.tile([128, 8, 128], F32R, tag="wstream")
            nc.sync.dma_start(
                out=wt,
                in_=wo_d.rearrange("(j p) o -> p j o", p=128)[
                    :, :, 128 * i : 128 * i + 128
                ],
            )
            ot = p_out.tile([128, 1024], F32, tag="ot", name="ot")
            for c in range(2):
                ps = pp.tile([128, 512], F32, tag="projv", bufs=2, name="ps5")
                for j in range(8):
                    _mm(nc, ps, wt[:, j, :], at[j][c],
                        start=(j == 0), stop=(j == 7))
                nc.vector.tensor_scalar_add(
                    ot[:, 512 * c : 512 * c + 512], ps, bo_sb[:, i : i + 1]
                )
            nc.sync.dma_start(out=out_d[128 * i : 128 * i + 128, :], in_=ot)


# ---------------------------------------------------------------------------
def _build_masks():
    j = np.arange(128)[:, None]
    r = np.arange(256)[None, :]
    M_B = (np.abs(j - r) <= 8).astype(np.float32)
    M_C = (np.abs(128 + j - r) <= 8).astype(np.float32)
    M_E = np.zeros((128, 256), np.float32)
    jj = np.arange(32)[:, None]
    M_E[0:32, :] = (r >= 248 + jj)
    jj = np.arange(32, 64)[:, None]
    M_E[32:64, :] = (r <= (jj + 64) - 120)
    M_A0 = np.zeros((128, 256), np.float32)
    M_A0[0:32, :] = M_E[32:64, :]
    return np.concatenate([M_B, M_C, M_E, M_A0], axis=1).astype(np.float32)


def _build_ind16():
    a = np.zeros((8, 16, 128), np.float32)
    for i in range(8):
        a[i, 2 * i, 0:64] = 1.0
        a[i, 2 * i + 1, 64:128] = 1.0
    return a


_NC_CACHE = {}


def _get_nc(phases=5):
    if phases not in _NC_CACHE:
        _NC_CACHE[phases] = _build_nc(phases)
    return _NC_CACHE[phases]


def _make_in_maps(inputs, n_cores):
    x = np.asarray(inputs["x"], dtype=np.float32)
    Wq = np.asarray(inputs["Wq"], dtype=np.float32)
    Wk = np.asarray(inputs["Wk"], dtype=np.float32)
    Wv = np.asarray(inputs["Wv"], dtype=np.float32)
    Wo = np.asarray(inputs["Wo"], dtype=np.float32)
    bq = np.asarray(inputs["bq"], dtype=np.float32)
    bk = np.asarray(inputs["bk"], dtype=np.float32)
    bv = np.asarray(inputs["bv"], dtype=np.float32)
    bo = np.asarray(inputs["bo"], dtype=np.float32)

    masks = _build_masks()
    ind16 = _build_ind16()
    ind_h = np.zeros((128, 2), np.float32)
    ind_h[0:64, 0] = 1.0
    ind_h[64:128, 1] = 1.0
    ones_c = np.ones((128, 1), np.float32)
    wqT = np.ascontiguousarray(Wq.T)
    wkT = np.ascontiguousarray(Wk.T)
    wvT = np.ascontiguousarray(Wv.T)
    woT = np.ascontiguousarray(Wo.T)
    bo_eff = (bo + Wo @ bv).astype(np.float32)

    in_maps = []
    for core in range(n_cores):
        subs = [2 * core, 2 * core + 1]
        Xc = np.concatenate([x[u // 4, u % 4 :: 4, :] for u in subs], 0)
        xT = np.ascontiguousarray(Xc.T)
        in_maps.append(
            {
                "xT": xT,
                "wqT": wqT,
                "wkT": wkT,
                "wvT": wvT,
                "woT": woT,
                "bq": bq,
                "bk": bk,
                "bo": bo_eff,
                "masks": masks,
                "ind16": ind16,
                "ind_h": ind_h,
                "ones_c": ones_c,
            }
        )
    return in_maps


def kernel(x, Wq, bq, Wk, bk, Wv, bv, Wo, bo, _cores=None):
    from concourse.bass_utils import run_bass_kernel_spmd

    x = np.asarray(x, dtype=np.float32)
    B, N, D = x.shape
    n_cores = N_CORES if _cores is None else _cores
    in_maps = _make_in_maps(
        dict(x=x, Wq=Wq, bq=bq, Wk=Wk, bk=bk, Wv=Wv, bv=bv, Wo=Wo, bo=bo), n_cores
    )
    nc = _get_nc()
    res = run_bass_kernel_spmd(nc, in_maps, core_ids=list(range(n_cores)))

    out = np.zeros((B, N, D), np.float32)
    for core in range(n_cores):
        oc = res.results[core]["outT"].T  # [t, o]
        for i, u in enumerate([2 * core, 2 * core + 1]):
            out[u // 4, u % 4 :: 4, :] = oc[512 * i : 512 * (i + 1)]
    return out

